# revision 27
# baseline (speedup 1.0000x reference)
"""DTM loss kernel for Trainium2 (8 NeuronCores, SPMD).

Math: for each of x_1, x_2 in [8192, 256]:
  D = cdist(x, x);  t[i] = sum of the 5 smallest entries of row i
loss = mean((t_1 - t_2)^2).

Sharding: cores 0-3 each take 2048 rows of x_1, cores 4-7 each take 2048
rows of x_2 (the program is identical, only the data differs).

Per core, for its rows i, the device computes e[i, j] = 2*x_i.x_j - sq_j
(top-8 of e per row == 8 smallest squared distances: sqrt is monotone and
sq_i is a per-row constant) and the DVE max8 instruction extracts the
per-group top-8 straight from PSUM. The -sq_j term is NOT a matmul pass:
the scalar engine pre-writes broadcast -sq_j rows into each PSUM group
and the bf16 feature matmuls (start=False) accumulate 2*x_i.x_j on top,
relying on PSUM has_written bits staying set from the previous use of
the bank (verified on hardware). This keeps the tensor engine at exactly
2 N=512 streams per 512-column chunk - the minimum for K=256 features.

The tiny [2048, 8] candidate lists return to the host, which forms
sq_i - e, clamps, takes sqrt, sums the 5 smallest and reduces the MSE.
"""

import sys

if "/opt/trn_rl_repo" not in sys.path:
    sys.path.insert(0, "/opt/trn_rl_repo")

import numpy as np

import concourse.bass as bass
import concourse.mybir as mybir
from concourse.bass_utils import run_bass_kernel_spmd
from concourse.tile import TileContext
from concourse.vector_clock import ScopedClock

N = 8192
D = 256
N_CORES = 8
ROWS = N * 2 // N_CORES  # 2048 rows per core (4 cores per matrix)
ROW_TILES = ROWS // 128  # 16 partition tiles per core
CHUNK = 512  # matmul moving free dim (half a PSUM group)
GRP_COLS = 1024  # columns per PSUM group (2 banks)
N_GRP = N // GRP_COLS  # 8 column groups per row-tile

F32 = mybir.dt.float32
F32R = mybir.dt.float32r
BF16 = mybir.dt.bfloat16

LAST_EXEC_TIME_NS = None
LAST_PROFILE = None


class FixedTileContext(TileContext):
    """TileContext legalized for a walrus that accepts only ONE embedded
    sync wait per instruction: extra waits are hoisted onto dedicated
    single-wait nops on the same engine."""

    def _commit_instruction(self, inst, lazy_reg_writes: bool = True):
        si = getattr(inst, "sync_info", None)
        waits = list(si.on_wait) if si is not None and si.on_wait else []
        if len(waits) > 1:
            engine = inst.engine
            for w in waits[:-1]:
                nop = mybir.InstNoOp(
                    name=self.nc.get_next_instruction_name(),
                    sync_info=mybir.SyncInfo(on_wait=[w], on_update=[]),
                    bass_nofuse=True,
                    engine=engine,
                )
                super()._commit_instruction(nop, lazy_reg_writes=False)
            inst.sync_info = mybir.SyncInfo(
                on_wait=[waits[-1]], on_update=list(si.on_update or [])
            )
        return super()._commit_instruction(inst, lazy_reg_writes=lazy_reg_writes)

    def _drain_and_barrier(self, tick_clock, wait_clock):
        drain_inst = self.nc.sync.drain()
        wait_clock.add_sem_waits(
            drain_inst.ins, ScopedClock({None: tick_clock.global_clock})
        )
        mi = drain_inst.ins
        si = mi.sync_info
        waits = list(si.on_wait) if si is not None and si.on_wait else []
        if len(waits) > 1:
            mi.sync_info = mybir.SyncInfo(
                on_wait=[waits[0]], on_update=list(si.on_update or [])
            )
            for w in waits[1:]:
                nop = self.nc.sync.nop(nofuse=True)
                nop.ins.sync_info = mybir.SyncInfo(on_wait=[w], on_update=[])
        self.nc.all_engine_barrier()
        assert self.sems is not None
        popped = self.nc._tile_sem_poison_stack.pop()
        assert popped is self._sem_poison
        # No second all_engine_barrier: the sem clears run on one engine's
        # stream, so NEFF completion (all streams done) still implies the
        # cleared state; nothing executes after them.
        self.nc.clear_and_free_semaphores(list(self.sems.allocated().values()))


_NC_CACHE = None


def _build_program():
    global _NC_CACHE
    if _NC_CACHE is not None:
        return _NC_CACHE

    nc = bass.Bass("TRN2", target_bir_lowering=False, debug=False,
                   num_devices=N_CORES)

    lhs_d = nc.dram_tensor("lhs", [D, ROWS], BF16, kind="ExternalInput")
    rhs_d = nc.dram_tensor("rhs", [D, N], BF16, kind="ExternalInput")
    srow_d = nc.dram_tensor("srow", [1, N], F32R, kind="ExternalInput")
    ones_d = nc.dram_tensor("ones", [1, 128], F32R, kind="ExternalInput")
    top_d = nc.dram_tensor("top", [ROWS, 8], F32, kind="ExternalOutput")

    EIGHTH = N // 8

    with FixedTileContext(nc) as tc:
        with (
            tc.tile_pool(name="rhs", bufs=1) as rhs_pool,
            tc.tile_pool(name="lhs", bufs=1) as lhs_pool,
            tc.tile_pool(name="sqb", bufs=1) as sqb_pool,
            tc.tile_pool(name="top", bufs=3) as top_pool,
            tc.tile_pool(name="ps", bufs=4, space="PSUM") as ps_pool,
        ):
            rhsA = rhs_pool.tile([128, N], BF16, tag="rhsA")
            rhsB = rhs_pool.tile([128, N], BF16, tag="rhsB")
            lhsA = lhs_pool.tile([128, ROWS], BF16, tag="lhsA")
            lhsB = lhs_pool.tile([128, ROWS], BF16, tag="lhsB")
            srow = lhs_pool.tile([1, N], F32R, tag="srow")
            onesr = lhs_pool.tile([1, 128], F32R, tag="onesr")
            sqb = sqb_pool.tile([128, N], BF16, tag="sqb")

            # Input DMAs across the three trigger engines (each a parallel
            # HW-DGE queue), in consumption order: pass 0 reads rhs cols
            # 0-2047 across all 16 row-tiles (~40 us of compute), so only
            # ~1 MB gates the first matmuls and the rest hides behind
            # compute. A and B chunk pieces interleave across queues.
            nc.sync.dma_start(out=srow[:], in_=srow_d[:])
            nc.gpsimd.dma_start(out=onesr[:], in_=ones_d[:])
            for c in range(4):
                cs = bass.ts(c, CHUNK)
                ea, eb = (nc.sync, nc.scalar) if c % 2 == 0 else (nc.scalar,
                                                                  nc.sync)
                ea.dma_start(out=rhsA[:, cs], in_=rhs_d[0:128, cs])
                eb.dma_start(out=rhsB[:, cs], in_=rhs_d[128:256, cs])
            for t in range(ROW_TILES):
                ps = bass.ts(t, 128)
                nc.gpsimd.dma_start(out=lhsA[:, ps], in_=lhs_d[0:128, ps])
                nc.gpsimd.dma_start(out=lhsB[:, ps], in_=lhs_d[128:256, ps])
            for q in range(2, 8):
                qs = bass.ts(q, EIGHTH)
                ea, eb = (nc.sync, nc.scalar) if q % 2 == 0 else (nc.scalar,
                                                                  nc.sync)
                ea.dma_start(out=rhsA[:, qs], in_=rhs_d[0:128, qs])
                eb.dma_start(out=rhsB[:, qs], in_=rhs_d[128:256, qs])

            # Build the [128, N] broadcast of -sq_j once (16 K=1 matmuls of
            # ones x srow + scalar evictions, hidden in the DMA window).
            # Side effect: every PSUM bank's has_written bits end up SET, so
            # the main loop's start=False matmuls accumulate onto whatever
            # the scalar engine pre-writes into the banks.
            for i in range(8):
                bp = ps_pool.tile([128, GRP_COLS], F32, tag="ps",
                                  name=f"ps_build_{i}")
                for h in range(2):
                    c = i * 2 + h
                    dst = bp[:, bass.ts(h, CHUNK)]
                    nc.tensor.matmul(dst, onesr[:],
                                     srow[:, bass.ts(c, CHUNK)],
                                     start=True, stop=True)
                    nc.scalar.activation(sqb[:, bass.ts(c, CHUNK)], dst,
                                         mybir.ActivationFunctionType.Copy)

            cands = [
                top_pool.tile([128, N_GRP * 8], F32, tag=f"cand{t}",
                              name=f"cand_{t}")
                for t in range(ROW_TILES)
            ]
            # Pass-major main loop: per [128, 1024] PSUM group (2 banks, 8
            # groups in flight), the scalar engine pre-writes -sq_j, four
            # bf16 matmuls accumulate 2*x_i.x_j, and the DVE max8 pulls the
            # group's top-8 straight from PSUM.
            for g in range(N_GRP):
                gs = bass.ts(g, GRP_COLS)
                for t in range(ROW_TILES):
                    ts = bass.ts(t, 128)
                    psum = ps_pool.tile([128, GRP_COLS], F32, tag="ps",
                                        name=f"ps_t{t}_g{g}")
                    nc.scalar.activation(psum[:], sqb[:, gs],
                                         mybir.ActivationFunctionType.Copy)
                    for ki, (lh, rh) in enumerate(((lhsA, rhsA),
                                                   (lhsB, rhsB))):
                        for c2 in range(2):
                            ch = g * 2 + c2
                            nc.tensor.matmul(
                                psum[:, bass.ts(c2, CHUNK)],
                                lh[:, ts],
                                rh[:, bass.ts(ch, CHUNK)],
                                start=False,
                                stop=(ki == 1 and c2 == 1),
                                skip_group_check=True,
                            )
                    nc.vector.max(out=cands[t][:, bass.ts(g, 8)],
                                  in_=psum[:])
                    if g == N_GRP - 1:
                        top = top_pool.tile([128, 8], F32, tag="top")
                        nc.vector.max(out=top[:], in_=cands[t][:])
                        nc.sync.dma_start(out=top_d[ts, :], in_=top[:])

    _NC_CACHE = nc
    return nc


def _self_distance_f32(x):
    """Per-row self 'distance' as the fp32 reference computes it:
    sqrt(max(0, 2*(||x||^2 - x.x))) with both terms rounded in fp32."""
    sq = np.sum(x * x, axis=1, dtype=np.float32)
    g = np.einsum("ij,ij->i", x, x, dtype=np.float32)
    d2 = np.float32(2.0) * (sq - g)
    return np.sqrt(np.maximum(d2, np.float32(0.0), dtype=np.float32),
                   dtype=np.float32)


def kernel(x_1, x_2, _trace=False):
    global LAST_EXEC_TIME_NS, LAST_PROFILE

    x_1 = np.ascontiguousarray(np.asarray(x_1, dtype=np.float32))
    x_2 = np.ascontiguousarray(np.asarray(x_2, dtype=np.float32))
    assert x_1.shape == (N, D) and x_2.shape == (N, D)

    import ml_dtypes

    nc = _build_program()

    host = {}
    for m, x in ((1, x_1), (2, x_2)):
        sq = np.sum(x * x, axis=1, dtype=np.float32)  # [N]
        xt = np.ascontiguousarray(x.T)  # [D, N]
        rhs = (2.0 * xt).astype(ml_dtypes.bfloat16)
        lhs = xt.astype(ml_dtypes.bfloat16)
        host[m] = (sq, rhs, lhs)

    ones = np.ones((1, 128), dtype=np.float32)
    in_maps = []
    for c in range(N_CORES):
        m = 1 if c < 4 else 2
        r0 = (c % 4) * ROWS
        in_maps.append({
            "lhs": np.ascontiguousarray(host[m][2][:, r0:r0 + ROWS]),
            "rhs": host[m][1],
            "srow": -host[m][0][None, :],
            "ones": ones,
        })

    res = run_bass_kernel_spmd(nc, in_maps, list(range(N_CORES)),
                               trace=_trace)
    LAST_EXEC_TIME_NS = res.exec_time_ns
    LAST_PROFILE = res.profile_json

    tops = {}
    for m, x, cores in ((1, x_1, range(0, 4)), (2, x_2, range(4, 8))):
        sq = host[m][0]
        e_top = np.concatenate(
            [res.results[c]["top"] for c in cores], axis=0
        )  # [N, 8] descending e values per row
        d2 = sq[:, None] - e_top.astype(np.float64)  # ascending squared dists
        # Column 0 is the self-match (squared distance ~ 0 up to fp noise,
        # 2+ orders of magnitude below any true neighbor). Replace it with
        # the same fp32-noise self term the reference produces, and sum the
        # next 4 true nearest neighbors.
        d_nn = np.sqrt(np.maximum(d2[:, 1:5], 0.0))
        tops[m] = d_nn.sum(axis=1) + _self_distance_f32(x)

    diff = tops[1] - tops[2]
    loss = np.mean(diff * diff)
    return np.float32(loss)
